# revision 10
# baseline (speedup 1.0000x reference)
"""3x3 neighborhood cosine-similarity sum (minus self) on 8 TRN2 NeuronCores.

Input:  input_image [1024, 1024, 1, C=128] float32  (H, W, 1, C)
Output: sim [1024, 1024] float32

sim = <xn, BoxSum3x3(xn)> - 1, xn = x / max(||x||, eps) per pixel.

Sharding: H rows split 128/core across 8 cores; each core receives 144 rows
(its 128 + 8-row aligned halo padding, zeros outside the image).

Per-core layout: w = 8p + j  ->  SBUF tiles [128 p, R=8 rows, 8 j, 128 c].
Each partition line is 4KB-contiguous in HBM (fast DMA); horizontal w+-1 is
a free-dim shift except at j=0/7 (handled by sub/super-diagonal matmuls).

Engine split per row batch (R=8 rows):
 - cast-DMA (SWDGE)  : f32 HBM -> fp16 SBUF
 - ACT               : sq = Square(xb); sqrt(ss+eps); S~ PSUM->SBUF evac
 - DVE               : ss = tensor_reduce(sq); inv = 1/sqrt; dup inv pairs;
                       xn = inv-broadcast * xb (4D pair-broadcast TT at 2x);
                       A = xn_{r-1}+xn_{r+1}; prod = xn*S~;
                       dot = tensor_reduce(prod); sim = dot - 1
 - PE                : S~ = sum_{dh in -1,0,1} shift_dh(A_r) + shift_dh(xn_r)
                       via identity matmuls w/ shifted rhs APs accumulated in
                       PSUM + sub/super-diagonal boundary matmuls
"""

import numpy as np
import ml_dtypes

import sys

for _p in ("/opt/trn_rl_repo",):
    if _p not in sys.path:
        sys.path.insert(0, _p)

import concourse.bass as bass
import concourse.bacc as bacc
import concourse.mybir as mybir
import concourse.tile as tile
from concourse.bass_utils import run_bass_kernel_spmd

F32 = mybir.dt.float32
BF16 = mybir.dt.bfloat16
F16 = mybir.dt.float16
ALU = mybir.AluOpType
ACTF = mybir.ActivationFunctionType
AXIS = mybir.AxisListType

H, W, C = 1024, 1024, 128
NCORES = 8
RPC = H // NCORES          # 128 output rows per core
R = 8                      # rows per batch
NJ = 8                     # j per partition; w = 8p + j
NB_OUT = RPC // R          # 16 output batches
NB_IN = NB_OUT + 2         # 18 input batches = 144 rows (8-row halo pad each side)
G = R * NJ                 # 64 (row, j) groups per batch


def build_consts():
    ident = np.eye(128, dtype=np.float32)
    subd = np.zeros((128, 128), np.float32)
    supd = np.zeros((128, 128), np.float32)
    for p in range(127):
        subd[p, p + 1] = 1.0   # out[m] += rhs[m-1]
        supd[p + 1, p] = 1.0   # out[m] += rhs[m+1]
    bf = lambda a: a.astype(np.float16)
    return bf(ident), bf(subd), bf(supd)


def build_bass():
    nc = bacc.Bacc(None, target_bir_lowering=False)
    x_dram = nc.declare_dram_parameter("x", [NB_IN * R, W, C], F32, isOutput=False)
    id_dram = nc.declare_dram_parameter("ident", [128, 128], F16, isOutput=False)
    sub_dram = nc.declare_dram_parameter("subd", [128, 128], F16, isOutput=False)
    sup_dram = nc.declare_dram_parameter("supd", [128, 128], F16, isOutput=False)
    out_dram = nc.declare_dram_parameter("out", [NB_OUT, 128, G], F32, isOutput=True)

    with tile.TileContext(nc) as tc:
        with (
            tc.tile_pool(name="consts", bufs=1) as cpool,
            tc.tile_pool(name="xb", bufs=2) as xpool,
            tc.tile_pool(name="sq", bufs=1) as sqpool,
            tc.tile_pool(name="xn", bufs=4) as xnpool,
            tc.tile_pool(name="aa", bufs=2) as apool,
            tc.tile_pool(name="ssb", bufs=2) as ssbpool,
            tc.tile_pool(name="prod", bufs=1) as prodpool,
            tc.tile_pool(name="stat", bufs=2) as statpool,
            tc.tile_pool(name="psum", bufs=2, space="PSUM") as psumpool,
        ):
            ident = cpool.tile([128, 128], F16, tag="ident")
            subd = cpool.tile([128, 128], F16, tag="subd")
            supd = cpool.tile([128, 128], F16, tag="supd")
            nc.sync.dma_start(ident[:], id_dram[:])
            nc.sync.dma_start(subd[:], sub_dram[:])
            nc.sync.dma_start(supd[:], sup_dram[:])
            eps = cpool.tile([128, 1], F32, tag="eps")
            nc.gpsimd.memset(eps[:], 1e-4)

            xn_t = [None] * NB_IN

            def fold_reduce(out_f32, t4d):
                """Sum over c (last axis, 128 wide) of t4d [128, R, NJ, C] fp16
                into out_f32 [128, G] via in-place 2x fold cascade."""
                v = t4d[:].rearrange("p r j c -> p (r j) c")
                w = C
                while w > 8:
                    h = w // 2
                    nc.vector.tensor_add(v[:, :, 0:h], v[:, :, 0:h], v[:, :, h:w])
                    w = h
                nc.vector.tensor_reduce(out_f32, v[:, :, 0:w], AXIS.X, ALU.add)


            def emit_output_batch(ob):
                xp, xc, xx = xn_t[ob], xn_t[ob + 1], xn_t[ob + 2]
                xn_t[ob] = None

                # A = xn_{r-1} + xn_{r+1} (vertical neighbors)
                A = apool.tile([128, R, NJ, C], F16, tag="A")
                nc.vector.tensor_add(A[:, 0], xp[:, R - 1], xc[:, 1])
                nc.vector.tensor_add(A[:, 1 : R - 1], xc[:, 0 : R - 2], xc[:, 2:R])
                nc.vector.tensor_add(A[:, R - 1], xc[:, R - 2], xx[:, 0])

                ssb = ssbpool.tile([128, R, NJ, C], F16, tag="ssb")
                for i2 in range(R // 2):
                    S2 = psumpool.tile([128, 2, NJ, C], F32, tag="S")
                    for ii in range(2):
                        i = 2 * i2 + ii
                        S = S2[:, ii]
                        Ar = A[:, i]
                        Xr = xc[:, i]
                        # A: dh=0 (starts both banks), dh=-1, dh=+1, boundaries
                        nc.tensor.matmul(S[:, 0:4], ident[:], Ar[:, 0:4], start=True, stop=False)
                        nc.tensor.matmul(S[:, 4:8], ident[:], Ar[:, 4:8], start=True, stop=False)
                        nc.tensor.matmul(S[:, 1:4], ident[:], Ar[:, 0:3], start=False, stop=False)
                        nc.tensor.matmul(S[:, 4:8], ident[:], Ar[:, 3:7], start=False, stop=False)
                        nc.tensor.matmul(S[:, 0:4], ident[:], Ar[:, 1:5], start=False, stop=False)
                        nc.tensor.matmul(S[:, 4:7], ident[:], Ar[:, 5:8], start=False, stop=False)
                        # xn_r: dh=-1, dh=+1
                        nc.tensor.matmul(S[:, 1:4], ident[:], Xr[:, 0:3], start=False, stop=False)
                        nc.tensor.matmul(S[:, 4:8], ident[:], Xr[:, 3:7], start=False, stop=False)
                        nc.tensor.matmul(S[:, 0:4], ident[:], Xr[:, 1:5], start=False, stop=False)
                        nc.tensor.matmul(S[:, 4:7], ident[:], Xr[:, 5:8], start=False, stop=False)
                        # boundary j=0 (w-1) and j=7 (w+1) for both A and xn
                        nc.tensor.matmul(S[:, 0:1], subd[:], Ar[:, 7:8], start=False, stop=False)
                        nc.tensor.matmul(S[:, 0:1], subd[:], Xr[:, 7:8], start=False, stop=False)
                        nc.tensor.matmul(S[:, 7:8], supd[:], Ar[:, 0:1], start=False, stop=False)
                        nc.tensor.matmul(S[:, 7:8], supd[:], Xr[:, 0:1], start=False, stop=False)
                        # xn_r dh=0 last, full width, carries stop
                        nc.tensor.matmul(S[:, 0:4], ident[:], Xr[:, 0:4], start=False, stop=True)
                        nc.tensor.matmul(S[:, 4:8], ident[:], Xr[:, 4:8], start=False, stop=True)

                    nc.scalar.activation(ssb[:, 2 * i2 : 2 * i2 + 2], S2[:], ACTF.Copy)

                prod = prodpool.tile([128, R, NJ, C], F16, tag="prod")
                h = R // 2
                # split prod (and its first fold) per half-batch so the first
                # half overlaps the second half's PSUM evacuations
                nc.vector.tensor_mul(prod[:, 0:h], xc[:, 0:h], ssb[:, 0:h])
                pv_a = prod[:, 0:h].rearrange("p r j c -> p (r j) c")
                nc.vector.tensor_add(pv_a[:, :, 0:64], pv_a[:, :, 0:64], pv_a[:, :, 64:128])
                nc.vector.tensor_mul(prod[:, h:R], xc[:, h:R], ssb[:, h:R])
                pv_b = prod[:, h:R].rearrange("p r j c -> p (r j) c")
                nc.vector.tensor_add(pv_b[:, :, 0:64], pv_b[:, :, 0:64], pv_b[:, :, 64:128])
                # remaining folds over the whole batch
                v = prod[:].rearrange("p r j c -> p (r j) c")
                w = 64
                while w > 8:
                    hh = w // 2
                    nc.vector.tensor_add(v[:, :, 0:hh], v[:, :, 0:hh], v[:, :, hh:w])
                    w = hh
                dotr = statpool.tile([128, G], F32, tag="dotr")
                nc.vector.tensor_reduce(dotr[:], v[:, :, 0:w], AXIS.X, ALU.add)
                nc.sync.dma_start(out_dram[ob], dotr[:])

            for b in range(NB_IN):
                xb = xpool.tile([128, R, NJ, C], F16, tag="xb")
                nc.gpsimd.dma_start(
                    xb[:],
                    x_dram[b * R : (b + 1) * R].rearrange("r (p j) c -> p r j c", p=128),
                )
                sq = sqpool.tile([128, R, NJ, C], F16, tag="sq")
                hb = R // 2
                nc.scalar.activation(sq[:, 0:hb], xb[:, 0:hb], ACTF.Square)
                sv_a = sq[:, 0:hb].rearrange("p r j c -> p (r j) c")
                nc.vector.tensor_add(sv_a[:, :, 0:64], sv_a[:, :, 0:64], sv_a[:, :, 64:128])
                nc.scalar.activation(sq[:, hb:R], xb[:, hb:R], ACTF.Square)
                sv_b = sq[:, hb:R].rearrange("p r j c -> p (r j) c")
                nc.vector.tensor_add(sv_b[:, :, 0:64], sv_b[:, :, 0:64], sv_b[:, :, 64:128])
                sv = sq[:].rearrange("p r j c -> p (r j) c")
                wq = 64
                while wq > 8:
                    hq = wq // 2
                    nc.vector.tensor_add(sv[:, :, 0:hq], sv[:, :, 0:hq], sv[:, :, hq:wq])
                    wq = hq
                ssr = statpool.tile([128, G], F32, tag="ssr")
                nc.vector.tensor_reduce(ssr[:], sv[:, :, 0:wq], AXIS.X, ALU.add)
                snorm = statpool.tile([128, G], F32, tag="snorm")
                nc.scalar.activation(snorm[:], ssr[:], ACTF.Sqrt, bias=eps[:])
                sinv = statpool.tile([128, G], F32, tag="sinv")
                nc.vector.reciprocal(sinv[:], snorm[:])
                invd = statpool.tile([128, G, 2], F16, tag="invd")
                nc.vector.tensor_scalar(invd[:, :, 0:1], sinv[:].unsqueeze(2), 1.0, None, ALU.mult)
                nc.vector.tensor_scalar(invd[:, :, 1:2], sinv[:].unsqueeze(2), 1.0, None, ALU.mult)

                xnb = xnpool.tile([128, R, NJ, C], F16, tag="xn")
                nc.vector.tensor_tensor(
                    xnb[:].rearrange("p r j (h two) -> p (r j) h two", two=2),
                    invd[:].unsqueeze(2).broadcast_to([128, G, C // 2, 2]),
                    xb[:].rearrange("p r j (h two) -> p (r j) h two", two=2),
                    ALU.mult,
                )
                xn_t[b] = xnb

                if b >= 2:
                    emit_output_batch(b - 2)

    nc.compile()
    return nc


def shard_inputs(input_image):
    """input_image [H, W, 1, C] f32 -> per-core in_maps (144 padded rows each)."""
    x = np.asarray(input_image).reshape(H, W, C).astype(np.float32, copy=False)
    ident, subd, supd = build_consts()
    in_maps = []
    for core in range(NCORES):
        lo = core * RPC
        shard = np.zeros((NB_IN * R, W, C), np.float32)
        # shard row i = global row (lo - 8 + i); valid range [lo-1, lo+128]
        gs = max(lo - 8, 0)
        ge = min(lo + RPC + 8, H)
        shard[gs - (lo - 8) : ge - (lo - 8)] = x[gs:ge]
        in_maps.append({"x": shard, "ident": ident, "subd": subd, "supd": supd})
    return in_maps


def unshard_output(results):
    """results[i]['out'] [NB_OUT, 128, G] -> [H, W] f32."""
    out = np.empty((H, W), np.float32)
    for core in range(NCORES):
        st = np.asarray(results[core]["out"]).reshape(NB_OUT, 128, R, NJ)
        sim = st.transpose(0, 2, 1, 3).reshape(RPC, W)  # w = 8p + j
        out[core * RPC : (core + 1) * RPC] = sim - 1.0
    return out


_NC_CACHE = {}


def get_nc():
    if "nc" not in _NC_CACHE:
        _NC_CACHE["nc"] = build_bass()
    return _NC_CACHE["nc"]


def kernel(input_image):
    nc = get_nc()
    in_maps = shard_inputs(input_image)
    res = run_bass_kernel_spmd(nc, in_maps, list(range(NCORES)))
    return unshard_output(res.results)


if __name__ == "__main__":
    rng = np.random.default_rng(0)
    x = rng.standard_normal((H, W, 1, C), dtype=np.float32)
    out = kernel(x)
    print(out.shape, out.dtype, out[:2, :4])


# revision 11
# speedup vs baseline: 1.2208x; 1.2208x over previous
"""3x3 neighborhood cosine-similarity sum (minus self) on 8 TRN2 NeuronCores.

Input:  input_image [1024, 1024, 1, C=128] float32  (H, W, 1, C)
Output: sim [1024, 1024] float32

sim = <xn, BoxSum3x3(xn)> - 1, xn = x / max(||x||, eps) per pixel.

Sharding: H rows split 128/core across 8 cores; each core receives 144 rows
(its 128 + 8-row aligned halo padding, zeros outside the image).

Per-core layout: w = 8p + j  ->  SBUF tiles [128 p, R=8 rows, 8 j, 128 c].
Each partition line is 4KB-contiguous in HBM (fast DMA); horizontal w+-1 is
a free-dim shift except at j=0/7 (handled by sub/super-diagonal matmuls).

Engine split per row batch (R=8 rows):
 - cast-DMA (SWDGE)  : f32 HBM -> fp16 SBUF
 - ACT               : sq = Square(xb); sqrt(ss+eps); S~ PSUM->SBUF evac
 - DVE               : ss = tensor_reduce(sq); inv = 1/sqrt; dup inv pairs;
                       xn = inv-broadcast * xb (4D pair-broadcast TT at 2x);
                       A = xn_{r-1}+xn_{r+1}; prod = xn*S~;
                       dot = tensor_reduce(prod); sim = dot - 1
 - PE                : S~ = sum_{dh in -1,0,1} shift_dh(A_r) + shift_dh(xn_r)
                       via identity matmuls w/ shifted rhs APs accumulated in
                       PSUM + sub/super-diagonal boundary matmuls
"""

import numpy as np
import ml_dtypes

import sys

for _p in ("/opt/trn_rl_repo",):
    if _p not in sys.path:
        sys.path.insert(0, _p)

import concourse.bass as bass
import concourse.bacc as bacc
import concourse.mybir as mybir
import concourse.tile as tile
from concourse.bass_utils import run_bass_kernel_spmd

F32 = mybir.dt.float32
BF16 = mybir.dt.bfloat16
F16 = mybir.dt.float16
ALU = mybir.AluOpType
ACTF = mybir.ActivationFunctionType
AXIS = mybir.AxisListType

H, W, C = 1024, 1024, 128
NCORES = 8
RPC = H // NCORES          # 128 output rows per core
R = 8                      # rows per batch
NJ = 8                     # j per partition; w = 8p + j
NB_OUT = RPC // R          # 16 output batches
NB_IN = NB_OUT + 2         # 18 input batches = 144 rows (8-row halo pad each side)
G = R * NJ                 # 64 (row, j) groups per batch


def build_consts():
    ident = np.eye(128, dtype=np.float32)
    subd = np.zeros((128, 128), np.float32)
    supd = np.zeros((128, 128), np.float32)
    for p in range(127):
        subd[p, p + 1] = 1.0   # out[m] += rhs[m-1]
        supd[p + 1, p] = 1.0   # out[m] += rhs[m+1]
    bf = lambda a: a.astype(np.float16)
    return bf(ident), bf(subd), bf(supd)


def build_bass():
    nc = bacc.Bacc(None, target_bir_lowering=False)
    x_dram = nc.declare_dram_parameter("x", [NB_IN * R, W, C], F32, isOutput=False)
    id_dram = nc.declare_dram_parameter("ident", [128, 128], F16, isOutput=False)
    sub_dram = nc.declare_dram_parameter("subd", [128, 128], F16, isOutput=False)
    sup_dram = nc.declare_dram_parameter("supd", [128, 128], F16, isOutput=False)
    out_dram = nc.declare_dram_parameter("out", [NB_OUT, 128, G], F32, isOutput=True)

    with tile.TileContext(nc) as tc:
        with (
            tc.tile_pool(name="consts", bufs=1) as cpool,
            tc.tile_pool(name="xb", bufs=2) as xpool,
            tc.tile_pool(name="sq", bufs=1) as sqpool,
            tc.tile_pool(name="xn", bufs=4) as xnpool,
            tc.tile_pool(name="aa", bufs=2) as apool,
            tc.tile_pool(name="ssb", bufs=2) as ssbpool,
            tc.tile_pool(name="prod", bufs=1) as prodpool,
            tc.tile_pool(name="stat", bufs=2) as statpool,
            tc.tile_pool(name="psum", bufs=2, space="PSUM") as psumpool,
        ):
            ident = cpool.tile([128, 128], F16, tag="ident")
            subd = cpool.tile([128, 128], F16, tag="subd")
            supd = cpool.tile([128, 128], F16, tag="supd")
            nc.sync.dma_start(ident[:], id_dram[:])
            nc.sync.dma_start(subd[:], sub_dram[:])
            nc.sync.dma_start(supd[:], sup_dram[:])
            eps = cpool.tile([128, 1], F32, tag="eps")
            nc.gpsimd.memset(eps[:], 1e-4)

            xn_t = [None] * NB_IN

            def fold_reduce(out_f32, t4d):
                """Sum over c (last axis, 128 wide) of t4d [128, R, NJ, C] fp16
                into out_f32 [128, G] via in-place 2x fold cascade."""
                v = t4d[:].rearrange("p r j c -> p (r j) c")
                w = C
                while w > 8:
                    h = w // 2
                    nc.vector.tensor_add(v[:, :, 0:h], v[:, :, 0:h], v[:, :, h:w])
                    w = h
                nc.vector.tensor_reduce(out_f32, v[:, :, 0:w], AXIS.X, ALU.add)


            def emit_output_batch(ob):
                xp, xc, xx = xn_t[ob], xn_t[ob + 1], xn_t[ob + 2]
                xn_t[ob] = None

                # A = xn_{r-1} + xn_{r+1} (vertical neighbors)
                A = apool.tile([128, R, NJ, C], F16, tag="A")
                nc.vector.tensor_add(A[:, 0], xp[:, R - 1], xc[:, 1])
                nc.vector.tensor_add(A[:, 1 : R - 1], xc[:, 0 : R - 2], xc[:, 2:R])
                nc.vector.tensor_add(A[:, R - 1], xc[:, R - 2], xx[:, 0])

                ssb = ssbpool.tile([128, R, NJ, C], F16, tag="ssb")
                for i2 in range(R // 2):
                    S2 = psumpool.tile([128, 2, NJ, C], F32, tag="S")
                    for ii in range(2):
                        i = 2 * i2 + ii
                        S = S2[:, ii]
                        Ar = A[:, i]
                        Xr = xc[:, i]
                        # A: dh=0 (starts both banks), dh=-1, dh=+1, boundaries
                        nc.tensor.matmul(S[:, 0:4], ident[:], Ar[:, 0:4], start=True, stop=False)
                        nc.tensor.matmul(S[:, 4:8], ident[:], Ar[:, 4:8], start=True, stop=False)
                        nc.tensor.matmul(S[:, 1:4], ident[:], Ar[:, 0:3], start=False, stop=False)
                        nc.tensor.matmul(S[:, 4:8], ident[:], Ar[:, 3:7], start=False, stop=False)
                        nc.tensor.matmul(S[:, 0:4], ident[:], Ar[:, 1:5], start=False, stop=False)
                        nc.tensor.matmul(S[:, 4:7], ident[:], Ar[:, 5:8], start=False, stop=False)
                        # xn_r: dh=-1, dh=+1
                        nc.tensor.matmul(S[:, 1:4], ident[:], Xr[:, 0:3], start=False, stop=False)
                        nc.tensor.matmul(S[:, 4:8], ident[:], Xr[:, 3:7], start=False, stop=False)
                        nc.tensor.matmul(S[:, 0:4], ident[:], Xr[:, 1:5], start=False, stop=False)
                        nc.tensor.matmul(S[:, 4:7], ident[:], Xr[:, 5:8], start=False, stop=False)
                        # boundary j=0 (w-1) and j=7 (w+1) for both A and xn
                        nc.tensor.matmul(S[:, 0:1], subd[:], Ar[:, 7:8], start=False, stop=False)
                        nc.tensor.matmul(S[:, 0:1], subd[:], Xr[:, 7:8], start=False, stop=False)
                        nc.tensor.matmul(S[:, 7:8], supd[:], Ar[:, 0:1], start=False, stop=False)
                        nc.tensor.matmul(S[:, 7:8], supd[:], Xr[:, 0:1], start=False, stop=False)
                        # xn_r dh=0 last, full width, carries stop
                        nc.tensor.matmul(S[:, 0:4], ident[:], Xr[:, 0:4], start=False, stop=True)
                        nc.tensor.matmul(S[:, 4:8], ident[:], Xr[:, 4:8], start=False, stop=True)

                    nc.scalar.activation(ssb[:, 2 * i2 : 2 * i2 + 2], S2[:], ACTF.Copy)

                prod = prodpool.tile([128, R, NJ, C], F16, tag="prod")
                h = R // 2
                # split prod (and its first fold) per half-batch so the first
                # half overlaps the second half's PSUM evacuations
                nc.vector.tensor_mul(prod[:, 0:h], xc[:, 0:h], ssb[:, 0:h])
                pv_a = prod[:, 0:h].rearrange("p r j c -> p (r j) c")
                nc.vector.tensor_add(pv_a[:, :, 0:64], pv_a[:, :, 0:64], pv_a[:, :, 64:128])
                nc.vector.tensor_mul(prod[:, h:R], xc[:, h:R], ssb[:, h:R])
                pv_b = prod[:, h:R].rearrange("p r j c -> p (r j) c")
                nc.vector.tensor_add(pv_b[:, :, 0:64], pv_b[:, :, 0:64], pv_b[:, :, 64:128])
                # remaining folds over the whole batch
                v = prod[:].rearrange("p r j c -> p (r j) c")
                w = 64
                while w > 8:
                    hh = w // 2
                    nc.vector.tensor_add(v[:, :, 0:hh], v[:, :, 0:hh], v[:, :, hh:w])
                    w = hh
                dotr = statpool.tile([128, G], F32, tag="dotr")
                nc.vector.tensor_reduce(dotr[:], v[:, :, 0:w], AXIS.X, ALU.add)
                sim = statpool.tile([128, G], F32, tag="sim")
                nc.vector.tensor_scalar(sim[:], dotr[:], -1.0, None, ALU.add)
                nc.sync.dma_start(out_dram[ob], sim[:])

            for b in range(NB_IN):
                xb = xpool.tile([128, R, NJ, C], F16, tag="xb")
                nc.gpsimd.dma_start(
                    xb[:],
                    x_dram[b * R : (b + 1) * R].rearrange("r (p j) c -> p r j c", p=128),
                )
                sq = sqpool.tile([128, R, NJ, C], F16, tag="sq")
                nc.scalar.activation(sq[:], xb[:], ACTF.Square)
                ssr = statpool.tile([128, G], F32, tag="ssr")
                fold_reduce(ssr[:], sq)
                snorm = statpool.tile([128, G], F32, tag="snorm")
                nc.scalar.activation(snorm[:], ssr[:], ACTF.Sqrt, bias=eps[:])
                sinv = statpool.tile([128, G], F32, tag="sinv")
                nc.vector.reciprocal(sinv[:], snorm[:])
                invd = statpool.tile([128, G, 2], F16, tag="invd")
                nc.vector.tensor_scalar(invd[:, :, 0:1], sinv[:].unsqueeze(2), 1.0, None, ALU.mult)
                nc.vector.tensor_scalar(invd[:, :, 1:2], sinv[:].unsqueeze(2), 1.0, None, ALU.mult)

                xnb = xnpool.tile([128, R, NJ, C], F16, tag="xn")
                nc.vector.tensor_tensor(
                    xnb[:].rearrange("p r j (h two) -> p (r j) h two", two=2),
                    invd[:].unsqueeze(2).broadcast_to([128, G, C // 2, 2]),
                    xb[:].rearrange("p r j (h two) -> p (r j) h two", two=2),
                    ALU.mult,
                )
                xn_t[b] = xnb

                if b >= 2:
                    emit_output_batch(b - 2)

    nc.compile()
    return nc


def shard_inputs(input_image):
    """input_image [H, W, 1, C] f32 -> per-core in_maps (144 padded rows each)."""
    x = np.asarray(input_image).reshape(H, W, C).astype(np.float32, copy=False)
    ident, subd, supd = build_consts()
    in_maps = []
    for core in range(NCORES):
        lo = core * RPC
        shard = np.zeros((NB_IN * R, W, C), np.float32)
        # shard row i = global row (lo - 8 + i); valid range [lo-1, lo+128]
        gs = max(lo - 8, 0)
        ge = min(lo + RPC + 8, H)
        shard[gs - (lo - 8) : ge - (lo - 8)] = x[gs:ge]
        in_maps.append({"x": shard, "ident": ident, "subd": subd, "supd": supd})
    return in_maps


def unshard_output(results):
    """results[i]['out'] [NB_OUT, 128, G] -> [H, W] f32."""
    out = np.empty((H, W), np.float32)
    for core in range(NCORES):
        st = np.asarray(results[core]["out"]).reshape(NB_OUT, 128, R, NJ)
        sim = st.transpose(0, 2, 1, 3).reshape(RPC, W)  # w = 8p + j
        out[core * RPC : (core + 1) * RPC] = sim
    return out


_NC_CACHE = {}


def get_nc():
    if "nc" not in _NC_CACHE:
        _NC_CACHE["nc"] = build_bass()
    return _NC_CACHE["nc"]


def kernel(input_image):
    nc = get_nc()
    in_maps = shard_inputs(input_image)
    res = run_bass_kernel_spmd(nc, in_maps, list(range(NCORES)))
    return unshard_output(res.results)


if __name__ == "__main__":
    rng = np.random.default_rng(0)
    x = rng.standard_normal((H, W, 1, C), dtype=np.float32)
    out = kernel(x)
    print(out.shape, out.dtype, out[:2, :4])
